# revision 1
# baseline (speedup 1.0000x reference)
"""RWKV WKV attention kernel for TRN2 (Bass/Tile), batch-parallel over 8 cores.

v2: software-pipelined x-prep (one chunk ahead), POOL offload (den/wkv/rwkv),
sigmoid via exp (single ACT table in steady state), Wv/Wr streamed from DRAM.

Per core (one batch element, x [T, D] fp32, D=1024):
  pass 1: transpose x -> [d, t]; time-mix; fp32r matmuls k,v,r; exp on ACT;
          native DVE tensor_tensor_scan for the WKV recurrence; rwkv -> DRAM.
  pass 2: out = rwkv^T @ Wo^T via fp32r matmuls, streamed from scratch.

Host-packed weights [128, 8*1024]: arr[p, j*1024+e] = W[e, j*128+p].
cv [128, 72] (col j of each group = channels j*128..j*128+127):
  0-7 mk, 8-15 mv, 16-23 mr, 24-31 1-mk, 32-39 1-mv, 40-47 1-mr,
  48-55 ew=exp(-exp(time_decay)), 56-63 u=time_first, 64-71 e^u.
"""
import sys
for p in ("/opt/trn_rl_repo",):
    if p not in sys.path:
        sys.path.insert(0, p)

import numpy as np
from contextlib import ExitStack

import concourse.bass as bass
import concourse.tile as tile
from concourse import bacc, mybir

dt = mybir.dt
AF = mybir.ActivationFunctionType
OP = mybir.AluOpType

D = 1024
NJ = D // 128  # 8 channel chunks


def build(nc, T=4096, TC=512):
    nch = T // TC
    NTS = TC // 128

    X = nc.dram_tensor("x", [T, D], dt.float32, kind="ExternalInput").ap()
    WK = nc.dram_tensor("wk", [128, NJ * D], dt.float32, kind="ExternalInput").ap()
    WV = nc.dram_tensor("wv", [128, NJ * D], dt.float32, kind="ExternalInput").ap()
    WR = nc.dram_tensor("wr", [128, NJ * D], dt.float32, kind="ExternalInput").ap()
    WO = nc.dram_tensor("wo", [128, NJ * D], dt.float32, kind="ExternalInput").ap()
    CV = nc.dram_tensor("cv", [128, 72], dt.float32, kind="ExternalInput").ap()
    IDT = nc.dram_tensor("ident", [128, 128], dt.float32, kind="ExternalInput").ap()
    O = nc.dram_tensor("o", [T, D], dt.float32, kind="ExternalOutput").ap()

    with tile.TileContext(nc) as tc, ExitStack() as octx:
        dram = octx.enter_context(tc.tile_pool(name="dram", bufs=nch, space="DRAM"))
        rwkvT = [dram.tile([D, TC], dt.float32r, tag=f"rwkvT{c}", name=f"rwkvT{c}")
                 for c in range(nch)]

        with ExitStack() as ctx:
            # ---------------- pass 1 ----------------
            wpool = ctx.enter_context(tc.tile_pool(name="wpool", bufs=1))
            wrs = ctx.enter_context(tc.tile_pool(name="wrs", bufs=13))
            xnp = ctx.enter_context(tc.tile_pool(name="xnp", bufs=4))
            tpp = ctx.enter_context(tc.tile_pool(name="tpp", bufs=1, space="PSUM"))
            xtp = ctx.enter_context(tc.tile_pool(name="xtp", bufs=NJ + 1))
            yp = ctx.enter_context(tc.tile_pool(name="yp", bufs=2))
            xmixp = ctx.enter_context(tc.tile_pool(name="xmixp", bufs=2 * NJ))
            xmrp = ctx.enter_context(tc.tile_pool(name="xmrp", bufs=NJ))
            kvps = ctx.enter_context(tc.tile_pool(name="kvps", bufs=2, space="PSUM"))
            vps = ctx.enter_context(tc.tile_pool(name="vps", bufs=3, space="PSUM"))
            rps = ctx.enter_context(tc.tile_pool(name="rps", bufs=2, space="PSUM"))
            ekp = ctx.enter_context(tc.tile_pool(name="ekp", bufs=2))
            ap_ = ctx.enter_context(tc.tile_pool(name="ap", bufs=2))
            sp = ctx.enter_context(tc.tile_pool(name="sp", bufs=2))
            ndp = ctx.enter_context(tc.tile_pool(name="ndp", bufs=2))
            wkvp = ctx.enter_context(tc.tile_pool(name="wkvp", bufs=NJ - 2))
            srp = ctx.enter_context(tc.tile_pool(name="srp", bufs=2))
            rwp = ctx.enter_context(tc.tile_pool(name="rwp", bufs=2))
            stp = ctx.enter_context(tc.tile_pool(name="stp", bufs=1))

            wk_t = wpool.tile([128, NJ * D], dt.float32r, tag="wk")
            nc.sync.dma_start(wk_t[:], WK.bitcast(dt.float32r))
            wv_t = wpool.tile([128, NJ * D], dt.float32r, tag="wv")
            nc.sync.dma_start(wv_t[:], WV.bitcast(dt.float32r))
            cv = wpool.tile([128, 72], dt.float32, tag="cv")
            nc.sync.dma_start(cv[:], CV)
            idt = wpool.tile([128, 128], dt.float32, tag="idt")
            nc.sync.dma_start(idt[:], IDT)

            def states(prefix):
                ts_ = []
                for j in range(NJ):
                    t = stp.tile([128, 1], dt.float32, tag=f"{prefix}{j}")
                    nc.vector.memset(t[:], 0.0)
                    ts_.append(t)
                return ts_

            xst = states("xst")
            ekst = states("ekst")
            ast = states("ast")
            alst = states("alst")
            best = states("best")

            def stage_load(c):
                """DMA x chunk + streamed Wv/Wr tiles (consumption order e,j)."""
                t0 = c * TC
                xn = []
                for s in range(NTS):
                    x_ = xnp.tile([128, D], dt.float32, tag="xn")
                    nc.sync.dma_start(x_[:], X[t0 + s * 128: t0 + (s + 1) * 128, :])
                    xn.append(x_)
                return xn

            def load_wrt_group(e, wrt):
                for j in range(NJ):
                    w = wrs.tile([128, 128], dt.float32r, tag="wrt")
                    nc.sync.dma_start(
                        w[:], WR[:, j * D + e * 128: j * D + (e + 1) * 128]
                        .bitcast(dt.float32r))
                    wrt[(j, e)] = w

            def stage_prep(c, xn):
                """Transpose + xT (halo col0) + time-mix for k and v."""
                xT = []
                for j in range(NJ):
                    tp = tpp.tile([128, TC], dt.float32, tag="tp")
                    for s in range(NTS):
                        nc.tensor.transpose(
                            tp[:, s * 128:(s + 1) * 128],
                            xn[s][:, j * 128:(j + 1) * 128], idt[:])
                    xt_ = xtp.tile([128, TC + 1], dt.float32, tag="xT")
                    nc.scalar.copy(xt_[:, 0:1], xst[j][:])
                    nc.scalar.copy(xt_[:, 1:TC + 1], tp[:])
                    nc.vector.tensor_copy(xst[j][:], xt_[:, TC:TC + 1])
                    xT.append(xt_)
                xmix = {}
                for pi, pname in ((0, "k"), (1, "v")):
                    for j in range(NJ):
                        y = yp.tile([128, TC], dt.float32, tag="y")
                        nc.scalar.activation(
                            y[:], xT[j][:, 0:TC], AF.Copy,
                            scale=cv[:, 24 + pi * 8 + j: 25 + pi * 8 + j])
                        xm = xmixp.tile([128, TC], dt.float32r, tag="xmix")
                        nc.vector.scalar_tensor_tensor(
                            xm[:], xT[j][:, 1:TC + 1],
                            cv[:, pi * 8 + j: pi * 8 + j + 1], y[:],
                            OP.mult, OP.add)
                        xmix[(pname, j)] = xm
                return xT, xmix

            def stage_kv(c, xmix):
                wkvs = []
                for e in range(NJ):
                    acck = kvps.tile([128, TC], dt.float32, tag="acck")
                    for j in range(NJ):
                        nc.tensor.matmul(
                            acck[:], wk_t[:, j * D + e * 128: j * D + (e + 1) * 128],
                            xmix[("k", j)][:], start=(j == 0), stop=(j == NJ - 1))
                    accv = vps.tile([128, TC], dt.float32, tag="accv")
                    for j in range(NJ):
                        nc.tensor.matmul(
                            accv[:], wv_t[:, j * D + e * 128: j * D + (e + 1) * 128],
                            xmix[("v", j)][:], start=(j == 0), stop=(j == NJ - 1))
                    # ACT: ek = exp(k), euk = exp(k + u)  (same Exp table)
                    ek = ekp.tile([128, TC + 1], dt.float32, tag="ek")
                    nc.vector.tensor_copy(ek[:, 0:1], ekst[e][:])
                    nc.scalar.activation(ek[:, 1:TC + 1], acck[:], AF.Exp)
                    euk = ndp.tile([128, TC], dt.float32, tag="euk")
                    nc.scalar.activation(euk[:], acck[:], AF.Exp,
                                         bias=cv[:, 56 + e: 57 + e])
                    nc.vector.tensor_copy(ekst[e][:], ek[:, TC:TC + 1])
                    # a = ek * v  (frees accv asap)
                    a = ap_.tile([128, TC + 1], dt.float32, tag="a")
                    nc.vector.tensor_copy(a[:, 0:1], ast[e][:])
                    nc.vector.tensor_tensor(a[:, 1:TC + 1], ek[:, 1:TC + 1], accv[:],
                                            OP.mult)
                    nc.vector.tensor_copy(ast[e][:], a[:, TC:TC + 1])
                    ewb = cv[:, 48 + e: 49 + e].broadcast_to([128, TC])
                    sa = sp.tile([128, TC], dt.float32, tag="sa")
                    nc.vector.tensor_tensor_scan(sa[:], ewb, a[:, 0:TC], alst[e][:],
                                                 OP.mult, OP.add)
                    nc.vector.tensor_copy(alst[e][:], sa[:, TC - 1:TC])
                    sb_ = sp.tile([128, TC], dt.float32, tag="sb")
                    nc.vector.tensor_tensor_scan(sb_[:], ewb, ek[:, 0:TC], best[e][:],
                                                 OP.mult, OP.add)
                    nc.vector.tensor_copy(best[e][:], sb_[:, TC - 1:TC])
                    eeu = cv[:, 64 + e: 65 + e]
                    num = ndp.tile([128, TC], dt.float32, tag="num")
                    nc.vector.scalar_tensor_tensor(num[:], a[:, 1:TC + 1], eeu, sa[:],
                                                   OP.mult, OP.add)
                    den = ndp.tile([128, TC], dt.float32, tag="den")
                    nc.gpsimd.tensor_tensor(den[:], euk[:], sb_[:], OP.add)
                    nc.vector.reciprocal_approx_fast(den[:], den[:])
                    wkv = wkvp.tile([128, TC], dt.float32, tag="wkv")
                    nc.gpsimd.tensor_tensor(wkv[:], num[:], den[:], OP.mult)
                    wkvs.append(wkv)
                return wkvs

            def stage_r_mix(c, xT):
                xmr = []
                for j in range(NJ):
                    y = yp.tile([128, TC], dt.float32, tag="y")
                    nc.scalar.activation(y[:], xT[j][:, 0:TC], AF.Copy,
                                         scale=cv[:, 40 + j: 41 + j])
                    xm = xmrp.tile([128, TC], dt.float32r, tag="xmr")
                    nc.vector.scalar_tensor_tensor(
                        xm[:], xT[j][:, 1:TC + 1], cv[:, 16 + j: 17 + j], y[:],
                        OP.mult, OP.add)
                    xmr.append(xm)
                return xmr

            def stage_r_mm(c, xmr, wrt, wkvs, wrt_next):
                for e in range(NJ):
                    accr = rps.tile([128, TC], dt.float32, tag="accr")
                    for j in range(NJ):
                        nc.tensor.matmul(accr[:], wrt[(j, e)][:], xmr[j][:],
                                         start=(j == 0), stop=(j == NJ - 1))
                    if wrt_next is not None:
                        load_wrt_group(e, wrt_next)
                    sr = srp.tile([128, TC], dt.float32, tag="sr")
                    nc.scalar.activation(sr[:], accr[:], AF.Sigmoid)
                    rw = rwp.tile([128, TC], dt.float32r, tag="rw")
                    nc.gpsimd.tensor_tensor(rw[:], wkvs[e][:], sr[:], OP.mult)
                    nc.gpsimd.dma_start(rwkvT[c][e * 128:(e + 1) * 128, :], rw[:])

            # ---- pipelined chunk loop ----
            xn = stage_load(0)
            wrt = {}
            for e in range(NJ):
                load_wrt_group(e, wrt)
            xT, xmix = stage_prep(0, xn)
            for c in range(nch):
                if c + 1 < nch:
                    xn_n = stage_load(c + 1)
                xmr = stage_r_mix(c, xT)
                wkvs = stage_kv(c, xmix)
                if c + 1 < nch:
                    xT_n, xmix_n = stage_prep(c + 1, xn_n)
                wrt_n = {} if c + 1 < nch else None
                stage_r_mm(c, xmr, wrt, wkvs, wrt_n)
                if c + 1 < nch:
                    xT, xmix, wrt = xT_n, xmix_n, wrt_n

        with ExitStack() as ctx:
            # ---------------- pass 2 ----------------
            wp2 = ctx.enter_context(tc.tile_pool(name="wp2", bufs=1))
            rwsp = ctx.enter_context(tc.tile_pool(name="rwsp", bufs=2 * NJ))
            ops_ = ctx.enter_context(tc.tile_pool(name="ops", bufs=4, space="PSUM"))
            ocp = ctx.enter_context(tc.tile_pool(name="ocp", bufs=4))

            wo_t = wp2.tile([128, NJ * D], dt.float32r, tag="wo")
            nc.sync.dma_start(wo_t[:], WO.bitcast(dt.float32r))

            for c in range(nch):
                t0 = c * TC
                rws = []
                for j in range(NJ):
                    rw = rwsp.tile([128, TC], dt.float32r, tag="rws")
                    nc.sync.dma_start(rw[:], rwkvT[c][j * 128:(j + 1) * 128, :])
                    rws.append(rw)
                for ts_ in range(NTS):
                    for eh in range(2):
                        op = ops_.tile([128, 512], dt.float32, tag="op")
                        for j in range(NJ):
                            nc.tensor.matmul(
                                op[:], rws[j][:, ts_ * 128:(ts_ + 1) * 128],
                                wo_t[:, j * D + eh * 512: j * D + (eh + 1) * 512],
                                start=(j == 0), stop=(j == NJ - 1))
                        oc = ocp.tile([128, 512], dt.float32, tag="oc")
                        nc.scalar.copy(oc[:], op[:])
                        nc.gpsimd.dma_start(
                            O[t0 + ts_ * 128: t0 + (ts_ + 1) * 128,
                              eh * 512:(eh + 1) * 512], oc[:])


def pack_inputs(x_slice, time_decay, time_first, time_mix_k, time_mix_v,
                time_mix_r, Wk, Wv, Wr, Wo):
    """Host-side packing for one core. x_slice: [T, D] fp32."""
    def packw(W):
        return np.ascontiguousarray(
            W.T.reshape(NJ, 128, D).transpose(1, 0, 2).reshape(128, NJ * D)
        ).astype(np.float32)

    def packv(v):
        return np.ascontiguousarray(v.reshape(NJ, 128).T).astype(np.float32)

    mk = time_mix_k.reshape(D).astype(np.float32)
    mv = time_mix_v.reshape(D).astype(np.float32)
    mr = time_mix_r.reshape(D).astype(np.float32)
    ew = np.exp(-np.exp(time_decay.astype(np.float32))).astype(np.float32)
    u = time_first.astype(np.float32).reshape(D)
    eu = np.exp(u).astype(np.float32)
    cv = np.concatenate([
        packv(mk), packv(mv), packv(mr),
        packv(1.0 - mk), packv(1.0 - mv), packv(1.0 - mr),
        packv(ew), packv(u), packv(eu)], axis=1).astype(np.float32)
    return {
        "x": np.ascontiguousarray(x_slice).astype(np.float32),
        "wk": packw(Wk), "wv": packw(Wv), "wr": packw(Wr), "wo": packw(Wo),
        "cv": cv, "ident": np.eye(128, dtype=np.float32),
    }


# ---------------------------------------------------------------------------
# Harness entry point: full inputs in, full output out, 8-way batch-parallel.
# ---------------------------------------------------------------------------
_CACHE = {}
_last_exec_time_ns = None


def _get_program(n_cores):
    key = ("prog", n_cores)
    if key not in _CACHE:
        nc = bacc.Bacc("TRN2", target_bir_lowering=False, debug=False,
                       num_devices=n_cores)
        build(nc, T=4096)
        nc.compile()
        _CACHE[key] = nc
    return _CACHE[key]


def kernel(x, time_decay, time_first, time_mix_k, time_mix_v, time_mix_r,
           Wk, Wv, Wr, Wo):
    """WKV attention: x [8, 4096, 1024] fp32 -> out [8, 4096, 1024] fp32.

    Shards batch across the 8 NeuronCores (one batch element per core).
    """
    global _last_exec_time_ns
    import os
    from concourse import bass_utils

    x = np.asarray(x, dtype=np.float32)
    B = x.shape[0]
    base = pack_inputs(x[0], np.asarray(time_decay), np.asarray(time_first),
                       np.asarray(time_mix_k), np.asarray(time_mix_v),
                       np.asarray(time_mix_r), np.asarray(Wk), np.asarray(Wv),
                       np.asarray(Wr), np.asarray(Wo))
    in_maps = []
    for b in range(B):
        m = dict(base)
        m["x"] = np.ascontiguousarray(x[b])
        in_maps.append(m)

    nc = _get_program(B)
    trace = os.environ.get("WKV_TRACE", "0") == "1"
    r = bass_utils.run_bass_kernel_spmd(nc, in_maps, core_ids=list(range(B)),
                                        trace=trace)
    _last_exec_time_ns = r.exec_time_ns
    return np.stack([r.results[b]["o"] for b in range(B)]).astype(np.float32)



# revision 14
# speedup vs baseline: 1.0948x; 1.0948x over previous
"""RWKV WKV attention kernel for TRN2 (Bass/Tile), batch-parallel over 8 cores.

v3: all-bf16 matmul path, single-pass (no DRAM scratch), DMA-transposed x
loads (no PE transposes), exp-rescaled recurrence (one ACT exp per step).

Per core (one batch element), chunked over TC=1024, e-groups of 4:
  mixes:  z = x_t - x_{t-1}; xm* = z*m_* + x_{t-1}        (DVE, bf16)
  GEMMs:  k/v/r = W_* @ xm*  (PE, bf16, [d,t] layout)
  wkv:    ek = exp(k) (ACT); a = ek*v (DVE); sa/sb = decaying scans over
          shifted a/ek (DVE + GpSimd); num = c*sa + a, den = c*sb + ek with
          c = exp(-u) (in-place); wkv = num/den; rw = (tanh(r/2)+1)*wkv
  out:    o = rw^T @ (0.5*Wo)^T  (PE), fp32 DMA straight from PSUM.

Host-packed weights [128, 8*1024] bf16: arr[p, j*1024+e] = W[e, j*128+p].
cv fp32 [128, 40] (col j of each group = channels j*128..j*128+127):
  0-7 mk, 8-15 mv, 16-23 mr, 24-31 ew=exp(-exp(time_decay)), 32-39 c=e^-u.
xp bf16 [16+T, D]: 16 zero rows then x (halo for the shifted time mix).
"""
import sys
for p in ("/opt/trn_rl_repo",):
    if p not in sys.path:
        sys.path.insert(0, p)

import numpy as np
from contextlib import ExitStack

import concourse.bass as bass
import concourse.tile as tile
from concourse import bacc, mybir

dt = mybir.dt
AF = mybir.ActivationFunctionType
OP = mybir.AluOpType

D = 1024
NJ = D // 128  # 8 channel chunks
GPSIMD_SCAN = False
STUB_EW = 0  # 0=full, 1=skip wkv/rw, 2=skip num..rw, 3=skip all
XPOSE_ENG = "sync"  # which HWDGE engine issues dma_start_transpose


def build(nc, T=4096, TC=1024):
    nch = T // TC
    H = 512           # PSUM-granularity half-tiles
    NH = TC // H
    EG = 4            # e-group size

    XP = nc.dram_tensor("xp", [16 + T, D], dt.bfloat16, kind="ExternalInput").ap()
    WK = nc.dram_tensor("wk", [128, NJ * D], dt.bfloat16, kind="ExternalInput").ap()
    WV = nc.dram_tensor("wv", [128, NJ * D], dt.bfloat16, kind="ExternalInput").ap()
    WR = nc.dram_tensor("wr", [128, NJ * D], dt.bfloat16, kind="ExternalInput").ap()
    WO = nc.dram_tensor("wo", [128, NJ * D], dt.bfloat16, kind="ExternalInput").ap()
    CV = nc.dram_tensor("cv", [128, 40], dt.float32, kind="ExternalInput").ap()
    O = nc.dram_tensor("o", [T, D], dt.bfloat16, kind="ExternalOutput").ap()

    with tile.TileContext(nc) as tc, ExitStack() as ctx:
        wpool = ctx.enter_context(tc.tile_pool(name="wpool", bufs=1))
        xtp = ctx.enter_context(tc.tile_pool(name="xtp", bufs=8))
        zp = ctx.enter_context(tc.tile_pool(name="zp", bufs=1))
        kxp = ctx.enter_context(tc.tile_pool(name="kxp", bufs=8))
        vxp = ctx.enter_context(tc.tile_pool(name="vxp", bufs=8))
        rxp = ctx.enter_context(tc.tile_pool(name="rxp", bufs=8))
        kvps = ctx.enter_context(tc.tile_pool(name="kvps", bufs=4, space="PSUM"))
        rps = ctx.enter_context(tc.tile_pool(name="rps", bufs=2, space="PSUM"))
        ops_ = ctx.enter_context(tc.tile_pool(name="ops", bufs=2, space="PSUM"))
        ekp = ctx.enter_context(tc.tile_pool(name="ekp", bufs=5))
        vcp = ctx.enter_context(tc.tile_pool(name="vcp", bufs=2))
        ap_ = ctx.enter_context(tc.tile_pool(name="ap", bufs=5))
        srp = ctx.enter_context(tc.tile_pool(name="srp", bufs=5))
        sap = ctx.enter_context(tc.tile_pool(name="sap", bufs=2))
        sbp = ctx.enter_context(tc.tile_pool(name="sbp", bufs=2))
        rwp = ctx.enter_context(tc.tile_pool(name="rwp", bufs=10))
        ocp = ctx.enter_context(tc.tile_pool(name="ocp", bufs=3))
        stp = ctx.enter_context(tc.tile_pool(name="stp", bufs=1))

        wk_t = wpool.tile([128, NJ * D], dt.bfloat16, tag="wk")
        nc.sync.dma_start(wk_t[:], WK)
        wv_t = wpool.tile([128, NJ * D], dt.bfloat16, tag="wv")
        nc.sync.dma_start(wv_t[:], WV)
        wr_t = wpool.tile([128, NJ * D], dt.bfloat16, tag="wr")
        nc.sync.dma_start(wr_t[:], WR)
        wo_t = wpool.tile([128, NJ * D], dt.bfloat16, tag="wo")
        nc.sync.dma_start(wo_t[:], WO)
        cv = wpool.tile([128, 40], dt.float32, tag="cv")
        nc.sync.dma_start(cv[:], CV)

        def states(prefix, dtype):
            ts_ = []
            for e in range(NJ):
                t = stp.tile([128, 1], dtype, tag=f"{prefix}{e}")
                nc.vector.memset(t[:], 0.0)
                ts_.append(t)
            return ts_

        ekst = states("ekst", dt.bfloat16)
        ast = states("ast", dt.bfloat16)
        alst = states("alst", dt.float32)
        best = states("best", dt.float32)

        def load_x(c):
            t0 = c * TC
            xts = []
            for j in range(NJ):
                xt = xtp.tile([128, TC + 16], dt.bfloat16, tag="xt")
                xeng = nc.sync if XPOSE_ENG == "sync" else nc.scalar
                xeng.dma_start_transpose(
                    xt[:], XP[t0: t0 + TC + 16, j * 128:(j + 1) * 128])
                xts.append(xt)
            return xts

        def mix_one(xts, pool, col0):
            """One projection's time-mix for all j: (x_t - x_prev)*m + x_prev."""
            ms = []
            for j in range(NJ):
                xt = xts[j]
                z = zp.tile([128, TC], dt.bfloat16, tag="z")
                nc.vector.tensor_tensor(
                    z[:], xt[:, 16:TC + 16], xt[:, 15:TC + 15], OP.subtract)
                m = pool.tile([128, TC], dt.bfloat16, tag="m")
                nc.vector.scalar_tensor_tensor(
                    m[:], z[:], cv[:, col0 + j: col0 + j + 1],
                    xt[:, 15:TC + 15], OP.mult, OP.add)
                ms.append(m)
            return ms

        def gemm(w_t, xm, e, h, pool):
            acc = pool.tile([128, H], dt.float32, tag="acc")
            for j in range(NJ):
                nc.tensor.matmul(
                    acc[:], w_t[:, j * D + e * 128: j * D + (e + 1) * 128],
                    xm[j][:, h * H:(h + 1) * H],
                    start=(j == 0), stop=(j == NJ - 1))
            return acc

        def out_gemm(prev):
            rws, c = prev
            t0 = c * TC
            for ts in range(TC // 128):
                for eh in range(D // H):
                    pso = ops_.tile([128, H], dt.float32, tag="pso")
                    for j in range(NJ):
                        nc.tensor.matmul(
                            pso[:], rws[j][:, ts * 128:(ts + 1) * 128],
                            wo_t[:, j * D + eh * H: j * D + (eh + 1) * H],
                            start=(j == 0), stop=(j == NJ - 1))
                    oc = ocp.tile([128, H], dt.bfloat16, tag="oc")
                    nc.scalar.copy(oc[:], pso[:])
                    nc.sync.dma_start(
                        O[t0 + ts * 128: t0 + (ts + 1) * 128,
                          eh * H:(eh + 1) * H], oc[:])

        # prologue: x + mixes for chunk 0
        xts = load_x(0)
        xmk = mix_one(xts, kxp, 0)
        xmv = mix_one(xts, vxp, 8)
        xmr = mix_one(xts, rxp, 16)
        prev_o = None  # (rws, c) pending output GEMM

        for c in range(nch):
            last = c + 1 >= nch
            if not last:
                xts_n = load_x(c + 1)
            rws_c = []
            for g in range(NJ // EG):
                es = range(g * EG, (g + 1) * EG)
                eks, as_, srs = {}, {}, {}
                # ---- k phase ----
                for e in es:
                    ek = ekp.tile([128, TC + 1], dt.bfloat16, tag="ek")
                    nc.vector.tensor_copy(ek[:, 0:1], ekst[e][:])
                    for h in range(NH):
                        acc = gemm(wk_t, xmk, e, h, kvps)
                        nc.scalar.activation(
                            ek[:, 1 + h * H: 1 + (h + 1) * H], acc[:], AF.Exp)
                    nc.vector.tensor_copy(ekst[e][:], ek[:, TC:TC + 1])
                    eks[e] = ek
                if g == 1 and not last:
                    xmk_n = mix_one(xts_n, kxp, 0)
                # ---- v phase (+ a = ek*v) ----
                for e in es:
                    vc = vcp.tile([128, TC], dt.bfloat16, tag="vc")
                    for h in range(NH):
                        acc = gemm(wv_t, xmv, e, h, kvps)
                        nc.scalar.copy(vc[:, h * H:(h + 1) * H], acc[:])
                    a = ap_.tile([128, TC + 1], dt.bfloat16, tag="a")
                    nc.vector.tensor_copy(a[:, 0:1], ast[e][:])
                    nc.gpsimd.tensor_tensor(
                        a[:, 1:TC + 1], eks[e][:, 1:TC + 1], vc[:], OP.mult)
                    nc.vector.tensor_copy(ast[e][:], a[:, TC:TC + 1])
                    as_[e] = a
                if g == 1 and not last:
                    xmv_n = mix_one(xts_n, vxp, 8)
                # ---- r phase ----
                for e in es:
                    sr = srp.tile([128, TC], dt.bfloat16, tag="sr")
                    for h in range(NH):
                        acc = gemm(wr_t, xmr, e, h, rps)
                        nc.scalar.activation(
                            sr[:, h * H:(h + 1) * H], acc[:], AF.Sigmoid)
                    srs[e] = sr
                if g == 1 and not last:
                    xmr_n = mix_one(xts_n, rxp, 16)
                # ---- wkv elementwise chain ----
                for e in es:
                    ek, a = eks[e], as_[e]
                    ewb = cv[:, 24 + e: 25 + e].broadcast_to([128, TC])
                    ce = cv[:, 32 + e: 33 + e]
                    sa = sap.tile([128, TC], dt.float32, tag="sa")
                    nc.vector.tensor_tensor_scan(
                        sa[:], ewb, a[:, 0:TC], alst[e][:], OP.mult, OP.add)
                    nc.vector.tensor_copy(alst[e][:], sa[:, TC - 1:TC])
                    sb = sbp.tile([128, TC], dt.float32, tag="sb")
                    nc.vector.tensor_tensor_scan(
                        sb[:], ewb, ek[:, 0:TC], best[e][:], OP.mult, OP.add)
                    nc.vector.tensor_copy(best[e][:], sb[:, TC - 1:TC])
                    # num (in-place over sa), den (in-place over sb); AP
                    # scalars are DVE-only (Pool STT rejects scalar ptrs)
                    nc.vector.scalar_tensor_tensor(
                        sa[:], sa[:], ce, a[:, 1:TC + 1], OP.mult, OP.add)
                    nc.vector.scalar_tensor_tensor(
                        sb[:], sb[:], ce, ek[:, 1:TC + 1], OP.mult, OP.add)
                    nc.vector.reciprocal_approx_fast(sb[:], sb[:])
                    nc.gpsimd.tensor_tensor(sa[:], sa[:], sb[:], OP.mult)
                    rw = rwp.tile([128, TC], dt.bfloat16, tag="rw")
                    nc.gpsimd.tensor_tensor(rw[:], srs[e][:], sa[:], OP.mult)
                    rws_c.append(rw)
                if g == 0 and prev_o is not None:
                    out_gemm(prev_o)
            prev_o = (rws_c, c)
            if not last:
                xts, xmk, xmv, xmr = xts_n, xmk_n, xmv_n, xmr_n

        out_gemm(prev_o)


def pack_inputs(x_slice, time_decay, time_first, time_mix_k, time_mix_v,
                time_mix_r, Wk, Wv, Wr, Wo):
    """Host-side packing for one core. x_slice: [T, D] fp32."""
    import ml_dtypes
    bf16 = ml_dtypes.bfloat16

    def packw(W):
        return np.ascontiguousarray(
            W.T.reshape(NJ, 128, D).transpose(1, 0, 2).reshape(128, NJ * D)
        ).astype(bf16)

    def packv(v):
        return np.ascontiguousarray(v.reshape(NJ, 128).T).astype(np.float32)

    T = x_slice.shape[0]
    xp = np.zeros((16 + T, D), dtype=bf16)
    xp[16:] = x_slice.astype(bf16)

    mk = time_mix_k.reshape(D).astype(np.float32)
    mv = time_mix_v.reshape(D).astype(np.float32)
    mr = time_mix_r.reshape(D).astype(np.float32)
    ew = np.exp(-np.exp(time_decay.astype(np.float32))).astype(np.float32)
    u = time_first.astype(np.float32).reshape(D)
    cvals = np.concatenate([
        packv(mk), packv(mv), packv(mr),
        packv(ew), packv(np.exp(-u))], axis=1).astype(np.float32)
    return {
        "xp": xp,
        "wk": packw(Wk), "wv": packw(Wv), "wr": packw(Wr),
        "wo": packw(Wo),
        "cv": cvals,
    }


# ---------------------------------------------------------------------------
# Harness entry point: full inputs in, full output out, 8-way batch-parallel.
# ---------------------------------------------------------------------------
_CACHE = {}
_last_exec_time_ns = None


def _get_program(n_cores):
    key = ("prog", n_cores)
    if key not in _CACHE:
        nc = bacc.Bacc("TRN2", target_bir_lowering=False, debug=False,
                       num_devices=n_cores)
        build(nc, T=4096)
        nc.compile()
        _CACHE[key] = nc
    return _CACHE[key]


def kernel(x, time_decay, time_first, time_mix_k, time_mix_v, time_mix_r,
           Wk, Wv, Wr, Wo):
    """WKV attention: x [8, 4096, 1024] fp32 -> out [8, 4096, 1024] fp32.

    Shards batch across the 8 NeuronCores (one batch element per core).
    """
    global _last_exec_time_ns
    import os
    import ml_dtypes
    from concourse import bass_utils

    x = np.asarray(x, dtype=np.float32)
    B = x.shape[0]
    base = pack_inputs(x[0], np.asarray(time_decay), np.asarray(time_first),
                       np.asarray(time_mix_k), np.asarray(time_mix_v),
                       np.asarray(time_mix_r), np.asarray(Wk), np.asarray(Wv),
                       np.asarray(Wr), np.asarray(Wo))
    in_maps = []
    for b in range(B):
        m = dict(base)
        if b > 0:
            xp = np.zeros_like(base["xp"])
            xp[16:] = x[b].astype(ml_dtypes.bfloat16)
            m["xp"] = xp
        in_maps.append(m)

    nc = _get_program(B)
    trace = os.environ.get("WKV_TRACE", "0") == "1"
    r = bass_utils.run_bass_kernel_spmd(nc, in_maps, core_ids=list(range(B)),
                                        trace=trace)
    _last_exec_time_ns = r.exec_time_ns
    return np.stack([np.asarray(r.results[b]["o"]).astype(np.float32)
                     for b in range(B)])


# revision 15
# speedup vs baseline: 1.1310x; 1.0331x over previous
"""RWKV WKV attention kernel for TRN2 (Bass/Tile), batch-parallel over 8 cores.

v3: all-bf16 matmul path, single-pass (no DRAM scratch), DMA-transposed x
loads (no PE transposes), exp-rescaled recurrence (one ACT exp per step).

Per core (one batch element), chunked over TC=1024, e-groups of 4:
  mixes:  z = x_t - x_{t-1}; xm* = z*m_* + x_{t-1}        (DVE, bf16)
  GEMMs:  k/v/r = W_* @ xm*  (PE, bf16, [d,t] layout)
  wkv:    ek = exp(k) (ACT); a = ek*v (DVE); sa/sb = decaying scans over
          shifted a/ek (DVE + GpSimd); num = c*sa + a, den = c*sb + ek with
          c = exp(-u) (in-place); wkv = num/den; rw = (tanh(r/2)+1)*wkv
  out:    o = rw^T @ (0.5*Wo)^T  (PE), fp32 DMA straight from PSUM.

Host-packed weights [128, 8*1024] bf16: arr[p, j*1024+e] = W[e, j*128+p].
cv fp32 [128, 40] (col j of each group = channels j*128..j*128+127):
  0-7 mk, 8-15 mv, 16-23 mr, 24-31 ew=exp(-exp(time_decay)), 32-39 c=e^-u.
xp bf16 [16+T, D]: 16 zero rows then x (halo for the shifted time mix).
"""
import sys
for p in ("/opt/trn_rl_repo",):
    if p not in sys.path:
        sys.path.insert(0, p)

import numpy as np
from contextlib import ExitStack

import concourse.bass as bass
import concourse.tile as tile
from concourse import bacc, mybir

dt = mybir.dt
AF = mybir.ActivationFunctionType
OP = mybir.AluOpType

D = 1024
NJ = D // 128  # 8 channel chunks
GPSIMD_SCAN = False
STUB_EW = 0  # 0=full, 1=skip wkv/rw, 2=skip num..rw, 3=skip all
XPOSE_ENG = "sync"  # which HWDGE engine issues dma_start_transpose


def build(nc, T=4096, TC=1024):
    nch = T // TC
    H = 512           # PSUM-granularity half-tiles
    NH = TC // H
    EG = 4            # e-group size

    XP = nc.dram_tensor("xp", [16 + T, D], dt.bfloat16, kind="ExternalInput").ap()
    WK = nc.dram_tensor("wk", [128, NJ * D], dt.bfloat16, kind="ExternalInput").ap()
    WV = nc.dram_tensor("wv", [128, NJ * D], dt.bfloat16, kind="ExternalInput").ap()
    WR = nc.dram_tensor("wr", [128, NJ * D], dt.bfloat16, kind="ExternalInput").ap()
    WO = nc.dram_tensor("wo", [128, NJ * D], dt.bfloat16, kind="ExternalInput").ap()
    CV = nc.dram_tensor("cv", [128, 40], dt.float32, kind="ExternalInput").ap()
    CVH = nc.dram_tensor("cvh", [128, 40], dt.bfloat16, kind="ExternalInput").ap()
    O = nc.dram_tensor("o", [T, D], dt.bfloat16, kind="ExternalOutput").ap()

    with tile.TileContext(nc) as tc, ExitStack() as ctx:
        wpool = ctx.enter_context(tc.tile_pool(name="wpool", bufs=1))
        xtp = ctx.enter_context(tc.tile_pool(name="xtp", bufs=8))
        zp = ctx.enter_context(tc.tile_pool(name="zp", bufs=1))
        kxp = ctx.enter_context(tc.tile_pool(name="kxp", bufs=8))
        vxp = ctx.enter_context(tc.tile_pool(name="vxp", bufs=8))
        rxp = ctx.enter_context(tc.tile_pool(name="rxp", bufs=8))
        kvps = ctx.enter_context(tc.tile_pool(name="kvps", bufs=4, space="PSUM"))
        rps = ctx.enter_context(tc.tile_pool(name="rps", bufs=2, space="PSUM"))
        ops_ = ctx.enter_context(tc.tile_pool(name="ops", bufs=2, space="PSUM"))
        ekp = ctx.enter_context(tc.tile_pool(name="ekp", bufs=5))
        vcp = ctx.enter_context(tc.tile_pool(name="vcp", bufs=2))
        ap_ = ctx.enter_context(tc.tile_pool(name="ap", bufs=5))
        srp = ctx.enter_context(tc.tile_pool(name="srp", bufs=5))
        sap = ctx.enter_context(tc.tile_pool(name="sap", bufs=2))
        sbp = ctx.enter_context(tc.tile_pool(name="sbp", bufs=2))
        dnp = ctx.enter_context(tc.tile_pool(name="dnp", bufs=2))
        rwp = ctx.enter_context(tc.tile_pool(name="rwp", bufs=10))
        ocp = ctx.enter_context(tc.tile_pool(name="ocp", bufs=3))
        stp = ctx.enter_context(tc.tile_pool(name="stp", bufs=1))

        wk_t = wpool.tile([128, NJ * D], dt.bfloat16, tag="wk")
        nc.sync.dma_start(wk_t[:], WK)
        wv_t = wpool.tile([128, NJ * D], dt.bfloat16, tag="wv")
        nc.sync.dma_start(wv_t[:], WV)
        wr_t = wpool.tile([128, NJ * D], dt.bfloat16, tag="wr")
        nc.sync.dma_start(wr_t[:], WR)
        wo_t = wpool.tile([128, NJ * D], dt.bfloat16, tag="wo")
        nc.sync.dma_start(wo_t[:], WO)
        cv = wpool.tile([128, 40], dt.float32, tag="cv")
        nc.sync.dma_start(cv[:], CV)
        cvh = wpool.tile([128, 40], dt.bfloat16, tag="cvh")
        nc.sync.dma_start(cvh[:], CVH)

        def states(prefix, dtype):
            ts_ = []
            for e in range(NJ):
                t = stp.tile([128, 1], dtype, tag=f"{prefix}{e}")
                nc.vector.memset(t[:], 0.0)
                ts_.append(t)
            return ts_

        ekst = states("ekst", dt.bfloat16)
        ast = states("ast", dt.bfloat16)
        alst = states("alst", dt.float32)
        best = states("best", dt.float32)

        def load_x(c):
            t0 = c * TC
            xts = []
            for j in range(NJ):
                xt = xtp.tile([128, TC + 16], dt.bfloat16, tag="xt")
                xeng = nc.sync if XPOSE_ENG == "sync" else nc.scalar
                xeng.dma_start_transpose(
                    xt[:], XP[t0: t0 + TC + 16, j * 128:(j + 1) * 128])
                xts.append(xt)
            return xts

        def mix_one(xts, pool, col0):
            """One projection's time-mix for all j: (x_t - x_prev)*m + x_prev."""
            ms = []
            for j in range(NJ):
                xt = xts[j]
                z = zp.tile([128, TC], dt.bfloat16, tag="z")
                nc.vector.tensor_tensor(
                    z[:], xt[:, 16:TC + 16], xt[:, 15:TC + 15], OP.subtract)
                m = pool.tile([128, TC], dt.bfloat16, tag="m")
                nc.vector.scalar_tensor_tensor(
                    m[:], z[:], cvh[:, col0 + j: col0 + j + 1],
                    xt[:, 15:TC + 15], OP.mult, OP.add)
                ms.append(m)
            return ms

        def gemm(w_t, xm, e, h, pool):
            acc = pool.tile([128, H], dt.float32, tag="acc")
            for j in range(NJ):
                nc.tensor.matmul(
                    acc[:], w_t[:, j * D + e * 128: j * D + (e + 1) * 128],
                    xm[j][:, h * H:(h + 1) * H],
                    start=(j == 0), stop=(j == NJ - 1))
            return acc

        def out_gemm(prev):
            rws, c = prev
            t0 = c * TC
            for ts in range(TC // 128):
                for eh in range(D // H):
                    pso = ops_.tile([128, H], dt.float32, tag="pso")
                    for j in range(NJ):
                        nc.tensor.matmul(
                            pso[:], rws[j][:, ts * 128:(ts + 1) * 128],
                            wo_t[:, j * D + eh * H: j * D + (eh + 1) * H],
                            start=(j == 0), stop=(j == NJ - 1))
                    oc = ocp.tile([128, H], dt.bfloat16, tag="oc")
                    nc.scalar.copy(oc[:], pso[:])
                    nc.sync.dma_start(
                        O[t0 + ts * 128: t0 + (ts + 1) * 128,
                          eh * H:(eh + 1) * H], oc[:])

        # prologue: x + mixes for chunk 0
        xts = load_x(0)
        xmk = mix_one(xts, kxp, 0)
        xmv = mix_one(xts, vxp, 8)
        xmr = mix_one(xts, rxp, 16)
        prev_o = None  # (rws, c) pending output GEMM

        for c in range(nch):
            last = c + 1 >= nch
            if not last:
                xts_n = load_x(c + 1)
            rws_c = []
            for g in range(NJ // EG):
                es = range(g * EG, (g + 1) * EG)
                eks, as_, srs = {}, {}, {}
                # ---- k phase ----
                for e in es:
                    ek = ekp.tile([128, TC + 1], dt.bfloat16, tag="ek")
                    nc.vector.tensor_copy(ek[:, 0:1], ekst[e][:])
                    for h in range(NH):
                        acc = gemm(wk_t, xmk, e, h, kvps)
                        nc.scalar.activation(
                            ek[:, 1 + h * H: 1 + (h + 1) * H], acc[:], AF.Exp)
                    nc.vector.tensor_copy(ekst[e][:], ek[:, TC:TC + 1])
                    eks[e] = ek
                if g == 1 and not last:
                    xmk_n = mix_one(xts_n, kxp, 0)
                # ---- v phase (+ a = ek*v) ----
                for e in es:
                    vc = vcp.tile([128, TC], dt.bfloat16, tag="vc")
                    for h in range(NH):
                        acc = gemm(wv_t, xmv, e, h, kvps)
                        nc.scalar.copy(vc[:, h * H:(h + 1) * H], acc[:])
                    a = ap_.tile([128, TC + 1], dt.bfloat16, tag="a")
                    nc.vector.tensor_copy(a[:, 0:1], ast[e][:])
                    nc.gpsimd.tensor_tensor(
                        a[:, 1:TC + 1], eks[e][:, 1:TC + 1], vc[:], OP.mult)
                    nc.vector.tensor_copy(ast[e][:], a[:, TC:TC + 1])
                    as_[e] = a
                if g == 1 and not last:
                    xmv_n = mix_one(xts_n, vxp, 8)
                # ---- r phase ----
                for e in es:
                    sr = srp.tile([128, TC], dt.bfloat16, tag="sr")
                    for h in range(NH):
                        acc = gemm(wr_t, xmr, e, h, rps)
                        nc.scalar.activation(
                            sr[:, h * H:(h + 1) * H], acc[:], AF.Sigmoid)
                    srs[e] = sr
                if g == 1 and not last:
                    xmr_n = mix_one(xts_n, rxp, 16)
                # ---- wkv elementwise chain ----
                for e in es:
                    ek, a = eks[e], as_[e]
                    ewb = cvh[:, 24 + e: 25 + e].broadcast_to([128, TC])
                    ce = cvh[:, 32 + e: 33 + e]
                    sa = sap.tile([128, TC], dt.bfloat16, tag="sa")
                    nc.vector.tensor_tensor_scan(
                        sa[:], ewb, a[:, 0:TC], alst[e][:], OP.mult, OP.add)
                    nc.vector.tensor_copy(alst[e][:], sa[:, TC - 1:TC])
                    sb = sbp.tile([128, TC], dt.bfloat16, tag="sb")
                    nc.vector.tensor_tensor_scan(
                        sb[:], ewb, ek[:, 0:TC], best[e][:], OP.mult, OP.add)
                    nc.vector.tensor_copy(best[e][:], sb[:, TC - 1:TC])
                    # num (in-place over sa, bf16); den fp32 for the recip.
                    # AP scalars are DVE-only (Pool rejects scalar ptrs).
                    nc.vector.scalar_tensor_tensor(
                        sa[:], sa[:], ce, a[:, 1:TC + 1], OP.mult, OP.add)
                    den = dnp.tile([128, TC], dt.float32, tag="den")
                    nc.vector.scalar_tensor_tensor(
                        den[:], sb[:], ce, ek[:, 1:TC + 1], OP.mult, OP.add)
                    nc.vector.reciprocal_approx_fast(den[:], den[:])
                    nc.gpsimd.tensor_tensor(sa[:], sa[:], den[:], OP.mult)
                    rw = rwp.tile([128, TC], dt.bfloat16, tag="rw")
                    nc.gpsimd.tensor_tensor(rw[:], srs[e][:], sa[:], OP.mult)
                    rws_c.append(rw)
                if g == 0 and prev_o is not None:
                    out_gemm(prev_o)
            prev_o = (rws_c, c)
            if not last:
                xts, xmk, xmv, xmr = xts_n, xmk_n, xmv_n, xmr_n

        out_gemm(prev_o)


def pack_inputs(x_slice, time_decay, time_first, time_mix_k, time_mix_v,
                time_mix_r, Wk, Wv, Wr, Wo):
    """Host-side packing for one core. x_slice: [T, D] fp32."""
    import ml_dtypes
    bf16 = ml_dtypes.bfloat16

    def packw(W):
        return np.ascontiguousarray(
            W.T.reshape(NJ, 128, D).transpose(1, 0, 2).reshape(128, NJ * D)
        ).astype(bf16)

    def packv(v):
        return np.ascontiguousarray(v.reshape(NJ, 128).T).astype(np.float32)

    T = x_slice.shape[0]
    xp = np.zeros((16 + T, D), dtype=bf16)
    xp[16:] = x_slice.astype(bf16)

    mk = time_mix_k.reshape(D).astype(np.float32)
    mv = time_mix_v.reshape(D).astype(np.float32)
    mr = time_mix_r.reshape(D).astype(np.float32)
    ew = np.exp(-np.exp(time_decay.astype(np.float32))).astype(np.float32)
    u = time_first.astype(np.float32).reshape(D)
    cvals = np.concatenate([
        packv(mk), packv(mv), packv(mr),
        packv(ew), packv(np.exp(-u))], axis=1).astype(np.float32)
    return {
        "xp": xp,
        "wk": packw(Wk), "wv": packw(Wv), "wr": packw(Wr),
        "wo": packw(Wo),
        "cv": cvals, "cvh": cvals.astype(bf16),
    }


# ---------------------------------------------------------------------------
# Harness entry point: full inputs in, full output out, 8-way batch-parallel.
# ---------------------------------------------------------------------------
_CACHE = {}
_last_exec_time_ns = None


def _get_program(n_cores):
    key = ("prog", n_cores)
    if key not in _CACHE:
        nc = bacc.Bacc("TRN2", target_bir_lowering=False, debug=False,
                       num_devices=n_cores)
        build(nc, T=4096)
        nc.compile()
        _CACHE[key] = nc
    return _CACHE[key]


def kernel(x, time_decay, time_first, time_mix_k, time_mix_v, time_mix_r,
           Wk, Wv, Wr, Wo):
    """WKV attention: x [8, 4096, 1024] fp32 -> out [8, 4096, 1024] fp32.

    Shards batch across the 8 NeuronCores (one batch element per core).
    """
    global _last_exec_time_ns
    import os
    import ml_dtypes
    from concourse import bass_utils

    x = np.asarray(x, dtype=np.float32)
    B = x.shape[0]
    base = pack_inputs(x[0], np.asarray(time_decay), np.asarray(time_first),
                       np.asarray(time_mix_k), np.asarray(time_mix_v),
                       np.asarray(time_mix_r), np.asarray(Wk), np.asarray(Wv),
                       np.asarray(Wr), np.asarray(Wo))
    in_maps = []
    for b in range(B):
        m = dict(base)
        if b > 0:
            xp = np.zeros_like(base["xp"])
            xp[16:] = x[b].astype(ml_dtypes.bfloat16)
            m["xp"] = xp
        in_maps.append(m)

    nc = _get_program(B)
    trace = os.environ.get("WKV_TRACE", "0") == "1"
    r = bass_utils.run_bass_kernel_spmd(nc, in_maps, core_ids=list(range(B)),
                                        trace=trace)
    _last_exec_time_ns = r.exec_time_ns
    return np.stack([np.asarray(r.results[b]["o"]).astype(np.float32)
                     for b in range(B)])
